# revision 1
# baseline (speedup 1.0000x reference)
"""Nadaraya-Watson kernel regression (retrieval_knn) on 8 NeuronCores.

out[b,d] = sum_n y[n,d] * G((Xw[n,d]-Zw[b,d])/h) / sum_n G(...),
G(z) = exp(-z^2/2); Zw = mlp(x), Xw = mlp(calc_X).

Sharding: data-parallel over the query batch B (64 queries/core);
calc_X / calc_Y / weights replicated.

Per-core plan (all fp32):
  - MLP over calc_X on the PE (weights stationary, X.T streamed) ->
    u.T = Xw.T/h  [16, 8192]   (1/h folded into W2.T on the host)
  - query MLP -> v = Zw.T/h [16, 64]
  - main pass in a [(rep,d)=128 partitions, n=8192 free] layout:
    partition p = r*16+d holds query b=8g+r (g = loop tile) and dim d.
    u rows replicated 8x across partitions once (single DMA); for each
    of 8 query-groups g: z = u - v (tensor_scalar with per-partition v),
    w = exp(-z^2/2) on ACT with accum_out giving the denominator row-sum,
    numerator via scalar_tensor_tensor(w * y) with accum_out.
  - epilogue: reciprocal + multiply, single linear DMA to y_out.
"""
import sys
sys.path.insert(0, '/opt/trn_rl_repo')
import numpy as np
from concourse import bass, tile, bacc, mybir
from concourse.bass_utils import run_bass_kernel_spmd

F32 = mybir.dt.float32
AF = mybir.ActivationFunctionType
ALU = mybir.AluOpType

B, N, DIN, DMID, DOUT = 512, 8192, 128, 256, 16
NCORES = 8
BC = B // NCORES            # 64 queries per core
NG = BC // 8                # 8 query-groups; partition p = r*16+d, b = 8g+r
MMF = 512                   # fp32 matmul moving-operand free-size limit
HCH = 2048                  # psum chunk (4 banks)


def build_kernel(reps=1):
    nc = bacc.Bacc(None, target_bir_lowering=False)

    NSL = N // NCORES       # per-core n-slice for the sharded reference MLP
    xT_d = nc.dram_tensor("xT", [DIN, BC], F32, kind="ExternalInput")
    XT_d = nc.dram_tensor("XTs", [DIN, NSL], F32, kind="ExternalInput")
    Y_d = nc.dram_tensor("calc_YT", [DOUT, N], F32, kind="ExternalInput")
    W1T_d = nc.dram_tensor("W1T", [DIN, DMID], F32, kind="ExternalInput")
    W2Ta_d = nc.dram_tensor("W2Ta", [DIN, DOUT], F32, kind="ExternalInput")
    W2Tb_d = nc.dram_tensor("W2Tb", [DIN, DOUT], F32, kind="ExternalInput")
    out_d = nc.dram_tensor("y_out", [BC, DOUT], F32, kind="ExternalOutput")

    with tile.TileContext(nc) as tc:
      for _rep in range(reps):
        with (
            tc.tile_pool(name="dram", bufs=1, space="DRAM") as dram,
            tc.tile_pool(name="const", bufs=1) as cpool,
        ):
            mlp_cm = tc.tile_pool(name="mlppool", bufs=1)
            mlp_pool = mlp_cm.__enter__()
            psum_cm = tc.tile_pool(name="ppsum", bufs=2, space="PSUM")
            psum = psum_cm.__enter__()

            # ---------- loads ----------
            XT = mlp_pool.tile([DIN, NSL], F32)
            nc.sync.dma_start(XT[:], XT_d[:])
            W1T = cpool.tile([DIN, DMID], F32)
            nc.sync.dma_start(W1T[:], W1T_d[:])
            W2Ta = cpool.tile([DIN, DOUT], F32)
            nc.sync.dma_start(W2Ta[:], W2Ta_d[:])
            W2Tb = cpool.tile([DIN, DOUT], F32)
            nc.sync.dma_start(W2Tb[:], W2Tb_d[:])
            xT = cpool.tile([DIN, BC], F32)
            nc.sync.dma_start(xT[:], xT_d[:])

            # ---------- query MLP: vT [16, 64] = Zw.T / h ----------
            pq = psum.tile([128, HCH], F32, tag="ph")
            for j in range(2):
                nc.tensor.matmul(pq[:, 64 * j:64 * j + 64],
                                 W1T[:, 128 * j:128 * j + 128], xT[:])
            HqT = cpool.tile([128, 128], F32)
            nc.scalar.activation(HqT[:], pq[:, 0:128], AF.Relu)
            pz = psum.tile([128, HCH], F32, tag="ph")
            nc.tensor.matmul(pz[0:DOUT, 0:BC], W2Ta[:], HqT[:, 0:64],
                             start=True, stop=False)
            nc.tensor.matmul(pz[0:DOUT, 0:BC], W2Tb[:], HqT[:, 64:128],
                             start=False, stop=True)
            vT = cpool.tile([DOUT, BC], F32)
            nc.scalar.activation(vT[:], pz[0:DOUT, 0:BC], AF.Copy)
            # store vT pre-arranged: vcol_dram[16r+d, g] = vT[d, 8g+r]
            vcol_dram = dram.tile([128, NG], F32)
            nc.sync.dma_start(
                bass.AP(vcol_dram[:].tensor, 0,
                        [[8, DOUT], [1, NG], [16 * NG, 8]]),
                vT[:].rearrange("d (g r) -> d g r", g=NG))
            v_col = cpool.tile([128, NG], F32)
            nc.sync.dma_start(v_col[:], vcol_dram[:])

            # ---------- sharded reference MLP: this core's n-slice ----------
            # u_slice [16, NSL], AllGather -> u_dram [16, 8192]
            HT = mlp_pool.tile([DIN, 2, NSL], F32)
            for j in range(2):
                ph = psum.tile([128, HCH], F32, tag="ph")
                for q in range(NSL // MMF):
                    nc.tensor.matmul(
                        ph[:, MMF * q:MMF * (q + 1)],
                        W1T[:, 128 * j:128 * j + 128],
                        XT[:, MMF * q:MMF * (q + 1)])
                dst = HT[:, j, :]
                if j % 2 == 0:
                    nc.scalar.activation(dst, ph[:, 0:NSL], AF.Relu)
                else:
                    nc.vector.tensor_scalar_max(dst, ph[:, 0:NSL], 0.0)

            u_sb = mlp_pool.tile([DOUT, NSL], F32)
            pu = psum.tile([128, HCH], F32, tag="ph")
            for q in range(NSL // MMF):
                dstw = slice(MMF * q, MMF * (q + 1))
                nc.tensor.matmul(pu[0:DOUT, dstw], W2Ta[:], HT[:, 0, dstw],
                                 start=True, stop=False)
                nc.tensor.matmul(pu[0:DOUT, dstw], W2Tb[:], HT[:, 1, dstw],
                                 start=False, stop=True)
            nc.scalar.activation(u_sb[:], pu[0:DOUT, 0:NSL], AF.Copy)
            psum_cm.__exit__(None, None, None)
            u_bounce = dram.tile([DOUT, NSL], F32)
            nc.sync.dma_start(u_bounce[:], u_sb[:])
            u_gath = dram.tile([NCORES * DOUT, NSL], F32)
            nc.gpsimd.collective_compute(
                "AllGather", ALU.bypass,
                replica_groups=[list(range(NCORES))],
                ins=[u_bounce[:].opt()],
                outs=[u_gath[:].opt()])
            # relayout [c, d, m] -> u_dram[d, c*NSL+m]
            u_dram = dram.tile([DOUT, N], F32)
            nc.sync.dma_start(
                bass.AP(u_dram[:].tensor, 0, [[N, DOUT], [NSL, NCORES], [1, NSL]]),
                bass.AP(u_gath[:].tensor, 0, [[NSL, DOUT], [DOUT * NSL, NCORES], [1, NSL]]))
            mlp_cm.__exit__(None, None, None)

            # ---------- main pass ----------
            den = cpool.tile([128, NG], F32)
            num = cpool.tile([128, NG], F32)
            with tc.tile_pool(name="mp", bufs=1) as mp:
                # U[16r+d, n] = u[d, n]  (one replicated load for all groups)
                U = mp.tile([128, N], F32, tag="U")
                nc.sync.dma_start(
                    U[:], bass.AP(u_dram[:].tensor, 0,
                                  [[0, 8], [N, DOUT], [1, N]]))
                # Yrep[16r+d, n] = y[n, d]
                Yrep = mp.tile([128, N], F32, tag="Yr")
                nc.sync.dma_start(
                    Yrep[:], bass.AP(Y_d[:].tensor, 0,
                                     [[0, 8], [N, DOUT], [1, N]]))
                # USQH[16r+d, n] = -u[d, n]^2/2 (computed on-chip)
                USQH = mp.tile([128, N], F32, tag="Uq")
                nc.vector.scalar_tensor_tensor(USQH[:], U[:], -0.5, U[:],
                                               op0=ALU.mult, op1=ALU.mult)
                for g in range(NG):
                    # arg = u*v - u^2/2  (e^{-v^2/2} factor cancels in the ratio)
                    sq = mp.tile([128, N], F32, tag="sq")
                    nc.vector.scalar_tensor_tensor(sq[:], U[:], v_col[:, g:g + 1],
                                                   USQH[:], op0=ALU.mult,
                                                   op1=ALU.add)
                    w = mp.tile([128, N], F32, tag="w", bufs=2)
                    nc.scalar.activation(w[:], sq[:], AF.Exp,
                                         accum_out=den[:, g:g + 1])
                    wy = mp.tile([128, N], F32, tag="sq")
                    nc.vector.scalar_tensor_tensor(wy[:], w[:], 1.0, Yrep[:],
                                                   op0=ALU.bypass, op1=ALU.mult,
                                                   accum_out=num[:, g:g + 1])

            # ---------- epilogue ----------
            rec = cpool.tile([128, NG], F32)
            nc.vector.reciprocal(rec[:], den[:])
            res = cpool.tile([128, NG], F32)
            nc.vector.tensor_mul(res[:], num[:], rec[:])
            # res[16r+d, g] -> y_out[8g+r, d]: flat idx = 128g + p
            nc.sync.dma_start(
                bass.AP(out_d[:].tensor, 0, [[1, 128], [128, NG]]), res[:])

    nc.compile()
    return nc


_NC = None


def prep_in_maps(inputs):
    x = np.asarray(inputs["x"], dtype=np.float32)
    calc_X = np.asarray(inputs["calc_X"], dtype=np.float32)
    calc_Y = np.ascontiguousarray(np.asarray(inputs["calc_Y"], dtype=np.float32))
    W1 = np.asarray(inputs["W1"], dtype=np.float32)
    W2 = np.asarray(inputs["W2"], dtype=np.float32)
    h = float(np.asarray(inputs["h"], dtype=np.float32).reshape(-1)[0])

    XT = np.ascontiguousarray(calc_X.T)                 # [128, 8192]
    YT = np.ascontiguousarray(calc_Y.T)                 # [16, 8192]
    W1T = np.ascontiguousarray(W1.T)                    # [128, 256]
    W2Th = np.ascontiguousarray(W2.T) / h               # [256, 16], 1/h folded
    W2Ta = np.ascontiguousarray(W2Th[0:128])
    W2Tb = np.ascontiguousarray(W2Th[128:256])

    NSL = N // NCORES
    in_maps = []
    for c in range(NCORES):
        xTc = np.ascontiguousarray(x[BC * c:BC * (c + 1)].T)   # [128, 64]
        XTs = np.ascontiguousarray(XT[:, NSL * c:NSL * (c + 1)])
        in_maps.append({
            "xT": xTc, "XTs": XTs, "calc_YT": YT,
            "W1T": W1T, "W2Ta": W2Ta, "W2Tb": W2Tb,
        })
    return in_maps


def kernel(**inputs):
    global _NC
    in_maps = prep_in_maps(inputs)
    if _NC is None:
        _NC = build_kernel()
    res = run_bass_kernel_spmd(_NC, in_maps, core_ids=list(range(NCORES)))
    out = np.concatenate([res.results[c]["y_out"] for c in range(NCORES)], axis=0)
    return out.astype(np.float32)


if __name__ == "__main__":
    rng = np.random.default_rng(0)
    ins = {
        "x": rng.standard_normal((B, DIN), dtype=np.float32),
        "calc_X": rng.standard_normal((N, DIN), dtype=np.float32),
        "calc_Y": rng.standard_normal((N, DOUT), dtype=np.float32),
        "W1": (rng.standard_normal((DMID, DIN), dtype=np.float32) * DIN ** -0.5),
        "W2": (rng.standard_normal((DOUT, DMID), dtype=np.float32) * DMID ** -0.5),
        "h": np.array([1.5], dtype=np.float32),
    }
    out = kernel(**ins)
    def mlp(v):
        return np.maximum(v @ ins["W1"].T, 0.0) @ ins["W2"].T
    Zw = mlp(ins["x"]); Xw = mlp(ins["calc_X"])
    z = (Xw[None] - Zw[:, None]) / ins["h"][0]
    w = np.exp(-0.5 * z * z)
    ref = (w * ins["calc_Y"][None]).sum(1) / w.sum(1)
    rel = np.abs(out - ref).max() / np.abs(ref).max()
    print("rel err:", rel)



# revision 3
# speedup vs baseline: 66.5054x; 66.5054x over previous
"""Nadaraya-Watson kernel regression via Mehler expansion on 8 NeuronCores.

out[b,d] = sum_n y[n,d] G((u[n,d]-v[b,d])) / sum_n G(...), G(z)=exp(-z^2/2),
u = mlp(calc_X)/h, v = mlp(x)/h.

Key identity: exp(-(u-v)^2/2) = sum_k phi_k(u) psi_k(v) with
phi_k(t) = exp(-t^2/2) t^k / sqrt(k!). Hence
  num[b,d] = sum_k psi_k(v[b,d]) S_k[d],  S_k[d] = sum_n phi_k(u[n,d]) y[n,d]
  den[b,d] = sum_k psi_k(v[b,d]) T_k[d],  T_k[d] = sum_n phi_k(u[n,d])
which collapses the B*N*D elementwise work to (N*D*R)/ncores + B*D*R.

Sharding: reference points n are sharded across the 8 cores (N/8 each);
queries b are sharded 64/core. Each core computes S/T partials for its
n-slice in a [128=(nh,d), 128] layout (accum_out columns), pre-reduces them
to [16, 2R] with one selector matmul, AllGathers the reduced partials
(~1.5KB, ~1us on HW), recombines with a second selector matmul, and runs the
tiny psi contraction for its own queries on 16 partitions, reading S/T
scalars straight from PSUM. MLP matmuls run in fp32r (4x fp32 throughput).
R=10 expansion terms; measured rel err vs fp32 reference ~1.5e-4.
"""
import sys
sys.path.insert(0, '/opt/trn_rl_repo')
import math
import numpy as np
from concourse import bass, tile, bacc, mybir
from concourse.bass_utils import run_bass_kernel_spmd

F32 = mybir.dt.float32
F32R = mybir.dt.float32r
AF = mybir.ActivationFunctionType
ALU = mybir.AluOpType

B, N, DIN, DMID, DOUT = 512, 8192, 128, 256, 16
NCORES = 8
BC = B // NCORES            # 64 queries per core
NSL = N // NCORES           # 1024 reference points per core
NF = NSL // 8               # 128 free elems in (nh,d) layout
R = 10                      # expansion terms
WPK = DMID + 2 * DOUT + BC  # wpack cols: W1T | W2Ta | W2Tb | xT
ISQ2 = float(1.0 / math.sqrt(2.0))


def build_kernel(reps=1):
    nc = bacc.Bacc(None, target_bir_lowering=False)

    XT_d = nc.dram_tensor("XTs", [DIN, NSL], F32R, kind="ExternalInput")
    Y_d = nc.dram_tensor("Y16", [DOUT, NSL], F32, kind="ExternalInput")
    WP_d = nc.dram_tensor("wpack", [128, WPK], F32R, kind="ExternalInput")
    sel_d = nc.dram_tensor("selw", [128, DOUT], F32, kind="ExternalInput")
    out_d = nc.dram_tensor("y_out", [BC, DOUT], F32, kind="ExternalOutput")

    with tile.TileContext(nc) as tc:
      for _rep in range(reps):
        with (
            tc.tile_pool(name="dram", bufs=1, space="DRAM") as dram,
            tc.tile_pool(name="sb", bufs=1) as sb,
            tc.tile_pool(name="psA", bufs=1, space="PSUM") as psA,
            tc.tile_pool(name="psB", bufs=3, space="PSUM") as psB,
        ):
            # ---------- loads ----------
            XT = sb.tile([DIN, NSL], F32R)
            nc.sync.dma_start(XT[:], XT_d[:])
            WP = sb.tile([128, WPK], F32R)
            nc.sync.dma_start(WP[:], WP_d[:])
            W1T = WP[:, 0:DMID]
            W2Ta = WP[:, DMID:DMID + DOUT]
            W2Tb = WP[:, DMID + DOUT:DMID + 2 * DOUT]
            xT = WP[:, DMID + 2 * DOUT:WPK]
            selt = sb.tile([128, DOUT], F32)
            nc.sync.dma_start(selt[:], sel_d[:])
            sel = selt[:]
            y_A = sb.tile([128, NF], F32)
            nc.sync.dma_start(
                y_A[:], bass.AP(Y_d[:].tensor, 0,
                                [[NF, 8], [NSL, DOUT], [1, NF]]))

            # ---------- reference MLP: u [16, NSL] = Xw.T / h ----------
            HT = sb.tile([DIN, 2, NSL], F32R)
            ph = psA.tile([128, 2048], F32, tag="mlp")
            for j in range(2):
                for q in range(2):
                    nc.tensor.matmul(
                        ph[:, 1024 * j + 512 * q:1024 * j + 512 * (q + 1)],
                        W1T[:, 128 * j:128 * (j + 1)],
                        XT[:, 512 * q:512 * (q + 1)])
                nc.scalar.activation(HT[:, j, :], ph[:, 1024 * j:1024 * (j + 1)],
                                     AF.Relu)
            u_dram = dram.tile([DOUT, NSL], F32)
            u_sb = sb.tile([DOUT, NSL], F32)
            pu = psB.tile([128, 512], F32, tag="sm")
            pu2 = psB.tile([128, 512], F32, tag="sm")
            for q, pp in ((0, pu), (1, pu2)):
                dstw = slice(512 * q, 512 * (q + 1))
                nc.tensor.matmul(pp[0:DOUT, 0:512], W2Ta[:], HT[:, 0, dstw],
                                 start=True, stop=False)
                nc.tensor.matmul(pp[0:DOUT, 0:512], W2Tb[:], HT[:, 1, dstw],
                                 start=False, stop=True)
                nc.scalar.activation(u_sb[:, dstw], pp[0:DOUT, 0:512], AF.Copy)
                nc.sync.dma_start(u_dram[:, dstw], u_sb[:, dstw])
            u_A = sb.tile([128, NF], F32)
            nc.sync.dma_start(
                u_A[:], bass.AP(u_dram[:].tensor, 0,
                                [[NF, 8], [NSL, DOUT], [1, NF]]))

            # ---------- phi recurrence -> S/T partial columns ----------
            ST = sb.tile([128, 2 * R], F32)
            phi = sb.tile([128, 2, NF], F32)
            usq = sb.tile([128, NF], F32)
            nc.scalar.activation(usq[:], u_A[:], AF.Square)
            nc.scalar.activation(phi[:, 0, :], usq[:], AF.Exp, scale=-0.5,
                                 accum_out=ST[:, R:R + 1])
            for k in range(R):
                wy = sb.tile([128, NF], F32, tag="wy", bufs=2)
                nc.vector.scalar_tensor_tensor(
                    wy[:], phi[:, k % 2, :], 1.0, y_A[:],
                    op0=ALU.bypass, op1=ALU.mult,
                    accum_out=ST[:, k:k + 1])
                if k < R - 1:
                    s = float(1.0 / math.sqrt(k + 1))
                    nc.vector.scalar_tensor_tensor(
                        phi[:, (k + 1) % 2, :], phi[:, k % 2, :], s, u_A[:],
                        op0=ALU.mult, op1=ALU.mult,
                        accum_out=ST[:, R + k + 1:R + k + 2])

            # pre-reduce own partials over nh: [128, 2R] -> [16, 2R]
            pown = psB.tile([128, 512], F32, tag="sm")
            nc.tensor.matmul(pown[0:DOUT, 0:2 * R], sel, ST[:])
            S16a = sb.tile([DOUT, 2 * R], F32)
            nc.scalar.activation(S16a[:], pown[0:DOUT, 0:2 * R], AF.Copy)
            st16_dram = dram.tile([DOUT, 2 * R], F32)
            nc.sync.dma_start(st16_dram[:], S16a[:])

            # ---------- AllGather reduced partials ----------
            st_gath = dram.tile([NCORES * DOUT, 2 * R], F32)
            nc.gpsimd.collective_compute(
                "AllGather", ALU.bypass,
                replica_groups=[list(range(NCORES))],
                ins=[st16_dram[:].opt()],
                outs=[st_gath[:].opt()])

            # ---------- query MLP (hidden under the gather) ----------
            pq = psB.tile([128, 512], F32, tag="sm")
            for j in range(2):
                nc.tensor.matmul(pq[:, 64 * j:64 * (j + 1)],
                                 W1T[:, 128 * j:128 * (j + 1)], xT)
            HqT = sb.tile([128, 128], F32R)
            nc.scalar.activation(HqT[:], pq[:, 0:128], AF.Relu)
            pz = psB.tile([128, 512], F32, tag="sm")
            nc.tensor.matmul(pz[0:DOUT, 0:BC], W2Ta[:], HqT[:, 0:64],
                             start=True, stop=False)
            nc.tensor.matmul(pz[0:DOUT, 0:BC], W2Tb[:], HqT[:, 64:128],
                             start=False, stop=True)
            vT = sb.tile([DOUT, BC], F32)
            nc.scalar.activation(vT[:], pz[0:DOUT, 0:BC], AF.Copy)

            # psi recurrence on [16, 64]
            psi = sb.tile([DOUT, R * BC], F32)
            vsq = sb.tile([DOUT, BC], F32)
            nc.scalar.activation(vsq[:], vT[:], AF.Square)
            nc.scalar.activation(psi[:, 0:BC], vsq[:], AF.Exp, scale=-0.5)
            for k in range(R - 1):
                s = float(1.0 / math.sqrt(k + 1))
                nc.vector.scalar_tensor_tensor(
                    psi[:, (k + 1) * BC:(k + 2) * BC],
                    psi[:, k * BC:(k + 1) * BC], s, vT[:],
                    op0=ALU.mult, op1=ALU.mult)

            # ---------- reduce gathered partials ----------
            stg = sb.tile([128, 2 * R], F32)
            nc.sync.dma_start(stg[:], st_gath[:])
            pst = psB.tile([128, 512], F32, tag="sm")
            nc.tensor.matmul(pst[0:DOUT, 0:2 * R], sel, stg[:])

            # ---------- contraction on [16, 64], scalars from PSUM ----------
            acc = sb.tile([DOUT, 2, 2, BC], F32)
            nc.vector.memset(acc[:, 0, 0, :], 0.0)
            nc.vector.memset(acc[:, 1, 0, :], 0.0)
            for k in range(R):
                pk = psi[:, k * BC:(k + 1) * BC]
                nc.vector.scalar_tensor_tensor(
                    acc[:, 0, (k + 1) % 2, :], pk, pst[0:DOUT, k:k + 1],
                    acc[:, 0, k % 2, :], op0=ALU.mult, op1=ALU.add)
                nc.vector.scalar_tensor_tensor(
                    acc[:, 1, (k + 1) % 2, :], pk, pst[0:DOUT, R + k:R + k + 1],
                    acc[:, 1, k % 2, :], op0=ALU.mult, op1=ALU.add)

            # ---------- epilogue: res [16, 64] -> y_out [64, 16] ----------
            rec = sb.tile([DOUT, BC], F32)
            nc.vector.reciprocal(rec[:], acc[:, 1, R % 2, :])
            res = sb.tile([DOUT, BC], F32)
            nc.vector.tensor_mul(res[:], acc[:, 0, R % 2, :], rec[:])
            nc.sync.dma_start(
                bass.AP(out_d[:].tensor, 0, [[1, DOUT], [DOUT, BC]]), res[:])

    nc.compile()
    return nc


def prep_in_maps(inputs):
    x = np.asarray(inputs["x"], dtype=np.float32)
    calc_X = np.asarray(inputs["calc_X"], dtype=np.float32)
    calc_Y = np.asarray(inputs["calc_Y"], dtype=np.float32)
    W1 = np.asarray(inputs["W1"], dtype=np.float32)
    W2 = np.asarray(inputs["W2"], dtype=np.float32)
    h = float(np.asarray(inputs["h"], dtype=np.float32).reshape(-1)[0])

    XT = np.ascontiguousarray(calc_X.T)                  # [128, 8192]
    YT = np.ascontiguousarray(calc_Y.T)                  # [16, 8192]
    W1T = np.ascontiguousarray(W1.T)                     # [128, 256]
    W2Th = np.ascontiguousarray(W2.T) / h                # [256, 16]
    p = np.arange(128)
    sel = (p[:, None] % 16 == np.arange(DOUT)[None, :]).astype(np.float32)

    in_maps = []
    for c in range(NCORES):
        sl = slice(NSL * c, NSL * (c + 1))
        wp = np.zeros((128, WPK), np.float32)
        wp[:, 0:DMID] = W1T
        wp[:, DMID:DMID + DOUT] = W2Th[0:128]
        wp[:, DMID + DOUT:DMID + 2 * DOUT] = W2Th[128:256]
        wp[:, DMID + 2 * DOUT:WPK] = x[BC * c:BC * (c + 1)].T
        in_maps.append({
            "XTs": np.ascontiguousarray(XT[:, sl]),
            "Y16": np.ascontiguousarray(YT[:, sl]),
            "wpack": wp, "selw": sel,
        })
    return in_maps


_NC = None


def kernel(**inputs):
    global _NC
    in_maps = prep_in_maps(inputs)
    if _NC is None:
        _NC = build_kernel()
    res = run_bass_kernel_spmd(_NC, in_maps, core_ids=list(range(NCORES)))
    out = np.concatenate([res.results[c]["y_out"] for c in range(NCORES)], axis=0)
    return out.astype(np.float32)


if __name__ == "__main__":
    rng = np.random.default_rng(0)
    ins = {
        "x": rng.standard_normal((B, DIN), dtype=np.float32),
        "calc_X": rng.standard_normal((N, DIN), dtype=np.float32),
        "calc_Y": rng.standard_normal((N, DOUT), dtype=np.float32),
        "W1": (rng.standard_normal((DMID, DIN), dtype=np.float32) * DIN ** -0.5),
        "W2": (rng.standard_normal((DOUT, DMID), dtype=np.float32) * DMID ** -0.5),
        "h": np.array([1.5], dtype=np.float32),
    }
    out = kernel(**ins)
    def mlp(v):
        return np.maximum(v @ ins["W1"].T, 0.0) @ ins["W2"].T
    Zw = mlp(ins["x"]); Xw = mlp(ins["calc_X"])
    z = (Xw[None] - Zw[:, None]) / ins["h"][0]
    w = np.exp(-0.5 * z * z)
    ref = (w * ins["calc_Y"][None]).sum(1) / w.sum(1)
    rel = np.abs(out - ref).max() / np.abs(ref).max()
    print("rel err:", rel)
